# revision 6
# baseline (speedup 1.0000x reference)
"""Multi-head causal attention (B=2,S=2048,D=768,H=12) on 8 NeuronCores.

Sharding: core = (batch, head_group) with 2 batches x 4 head groups of 3
heads.  Each core computes q/k/v projections for its heads, causal
attention, and a partial output projection (wo rows for its heads); the
host sums the 4 partials per batch and adds bo.

v2: host-side transpose/pre-pack of x and weights (dense per-partition
DMAs), fast approximate reciprocal, mask-muls on gpsimd, causal trimming
of diagonal-tile matmuls/exp, zero-bias fast path, interleaved
projection/attention emission, batched output DMA.
"""

import numpy as np

import bass_rust
import concourse.bass as bass
import concourse.mybir as mybir
import concourse.tile as tile
from concourse.bass_utils import run_bass_kernel_spmd

F16 = mybir.dt.float16
F32 = mybir.dt.float32

B, S, D = 2, 2048, 768
H, DK = 12, 64
HPC = 3            # heads per core
N_CORES = 8
QB = 512           # query block (psum free dim)
NQB = S // QB      # 4
NKT = S // 128     # 16 key tiles
DKT = D // 128     # 6 contraction tiles for projections
NST = S // 128     # 16 s-chunks

ScopedClock = bass_rust.ScopedClock


# ---------------------------------------------------------------------------
# walrus in this build accepts at most ONE sync-wait per instruction; spread
# extra waits onto NOPs placed immediately before the owning instruction.

def _split_drain_and_barrier(self, tick_clock, wait_clock):
    probe = self.nc.sync.nop()
    wait_clock.add_sem_waits(probe.ins, ScopedClock({None: tick_clock.global_clock}))
    si = probe.ins.sync_info
    waits = list(si.on_wait) if si is not None else []
    if len(waits) > 1:
        si.on_wait = waits[:1]
        for w in waits[1:]:
            n = self.nc.sync.nop()
            nsi = n.ins.sync_info
            if nsi is None:
                n.ins.sync_info = bass_rust.SyncInfo(on_wait=[w], on_update=[])
            else:
                nsi.on_wait = [w]
    self.nc.sync.drain()

    self.nc.all_engine_barrier()
    assert self.sems is not None
    popped = self.nc._tile_sem_poison_stack.pop()
    assert popped is self._sem_poison
    self.nc.clear_and_free_semaphores(list(self.sems.allocated().values()))
    self.nc.all_engine_barrier()


tile.TileContext._drain_and_barrier = _split_drain_and_barrier

_nop_ctr = [0]


def split_multi_waits(nc):
    def visit(parent):
        for bb in parent.blocks:
            insts = bb.instructions
            out = []
            changed = False
            for inst in insts:
                si = inst.sync_info
                if si is not None and len(si.on_wait) > 1:
                    waits = list(si.on_wait)
                    for w in waits[:-1]:
                        _nop_ctr[0] += 1
                        nop = mybir.InstNoOp(
                            name=f"wsplit{_nop_ctr[0]}",
                            sync_info=mybir.SyncInfo(on_wait=[w], on_update=[]),
                            bass_nofuse=True,
                            engine=inst.engine,
                        )
                        out.append(nop)
                    si.on_wait = waits[-1:]
                    changed = True
                out.append(inst)
            if changed:
                bb.instructions = out
    for f in nc.m.functions:
        visit(f)


# ---------------------------------------------------------------------------


SKIP = set()


def build_nc(causal: bool, zb: bool = False, timing: bool = False, nrep: int = 1):
    skip = SKIP
    nc = bass.Bass("TRN2", target_bir_lowering=False, debug=False,
                   num_devices=N_CORES)

    # host-transposed x: chunk-major [NQB, 128, DKT, QB]
    x16t = nc.dram_tensor("x16t", (NQB, 128, DKT, QB), F16,
                          kind="ExternalInput").ap()
    wqe = nc.dram_tensor("wqe", (128, DKT, 256), F16, kind="ExternalInput").ap()
    wke = nc.dram_tensor("wke", (128, DKT, 256), F16, kind="ExternalInput").ap()
    wve = nc.dram_tensor("wve", (128, DKT, HPC * DK), F16,
                         kind="ExternalInput").ap()
    woh2 = nc.dram_tensor("woh2", (DK, D), F16, kind="ExternalInput").ap()
    woe2 = nc.dram_tensor("woe2", (128, D), F16, kind="ExternalInput").ap()
    bqk = nc.dram_tensor("bqk", (128, 4), F32, kind="ExternalInput").ap()
    bvp = nc.dram_tensor("bvp", (DK, HPC), F32, kind="ExternalInput").ap()
    if not causal:
        mTd = nc.dram_tensor("mT", (S, S), F16, kind="ExternalInput").ap()
    if timing:
        o16 = nc.dram_tensor("o16", (S, D), F16, kind="Internal").ap()
        dummy = nc.dram_tensor("tout", (128, 4), F32, kind="ExternalOutput").ap()
    else:
        o16 = nc.dram_tensor("o16", (S, D), F16, kind="ExternalOutput").ap()

    with tile.TileContext(nc) as tc:
        cst = tc.alloc_tile_pool(name="cst", bufs=1)
        ps_gen = tc.alloc_tile_pool(name="psg", bufs=2, space="PSUM")
        ps_sc = tc.alloc_tile_pool(name="pss", bufs=2, space="PSUM")
        ps_pv = tc.alloc_tile_pool(name="psv", bufs=2, space="PSUM")
        pt_pool = tc.alloc_tile_pool(name="ptp", bufs=2)
        ctx_pool = tc.alloc_tile_pool(name="ctp", bufs=2)
        rs_pool = tc.alloc_tile_pool(name="rsp", bufs=2)
        out_pool = tc.alloc_tile_pool(name="outp", bufs=2)
        if not causal:
            msk_pool = tc.alloc_tile_pool(name="mskp", bufs=2)

        for _rep in range(nrep):
            # ---- constant loads (dense per-partition layouts, host-packed)
            wq_sb = cst.tile([128, DKT, 256], F16, tag="wq")
            wk_sb = cst.tile([128, DKT, 256], F16, tag="wk")
            wv_sb = cst.tile([128, DKT, HPC * DK], F16, tag="wv")
            wo_sb = cst.tile([DK, D], F16, tag="wo")
            wo_sb2 = cst.tile([128, D], F16, tag="wo2")
            nc.sync.dma_start(wq_sb[:], wqe)
            nc.sync.dma_start(wk_sb[:], wke)
            nc.sync.dma_start(wv_sb[:], wve)

            xT = cst.tile([128, NQB, DKT, QB], F16, tag="xT")
            if "xtd" in skip:
                nc.vector.memset(xT[:, 0, :, 0:8], 0.0)
            else:
                nc.sync.dma_start(xT[:, 0], x16t[0])

            nc.sync.dma_start(wo_sb[:], woh2)
            nc.sync.dma_start(wo_sb2[:], woe2)
            bqk_sb = cst.tile([128, 4], F32, tag="bqk")
            bv_sb = cst.tile([DK, HPC], F32, tag="bv")
            nc.sync.dma_start(bqk_sb[:], bqk)
            nc.sync.dma_start(bv_sb[:], bvp)
            if "xtd" not in skip:
                for ch in range(1, NQB):
                    nc.sync.dma_start(xT[:, ch], x16t[ch])

            tril = cst.tile([128, 128], F16, tag="tril")
            nc.gpsimd.memset(tril[:], 1.0)
            # keep (f - p >= 0) i.e. q >= k, else 0
            nc.gpsimd.affine_select(
                out=tril[:], in_=tril[:], compare_op=mybir.AluOpType.is_ge,
                fill=0.0, base=0, pattern=[[1, 128]], channel_multiplier=-1)

            ones = cst.tile([128, DK], F32, tag="ones")
            nc.vector.memset(ones[:], 1.0)
            if timing:
                nc.sync.dma_start(dummy, ones[:, 0:4])

            V_sb = cst.tile([128, NKT, HPC, DK + 1], F16, tag="V")
            # only the rowsum ones-column needs presetting
            nc.vector.memset(V_sb[:, :, :, DK:DK + 1], 1.0)

            qT0 = cst.tile([128, S], F16, tag="qT0")
            qT1 = cst.tile([128, S], F16, tag="qT1")
            kT0 = cst.tile([128, S], F16, tag="kT0")
            kT1 = cst.tile([128, S], F16, tag="kT1")

            for i in range(NQB):
                # ---- Q^T / K^T projections for this query block
                # (heads stacked on partitions)
                for wsb, dsts, bcol in ((wq_sb, (qT0, qT1), 0),
                                        (wk_sb, (kT0, kT1), 2)):
                    for t in range(2):
                        ps = ps_gen.tile([128, QB], F32, tag="psg")
                        for c in range(DKT):
                            nc.tensor.matmul(
                                ps[:], wsb[:, c, t * 128:(t + 1) * 128],
                                xT[:, i, c, :],
                                start=(c == 0), stop=(c == DKT - 1))
                        dst = dsts[t][:, i * QB:(i + 1) * QB]
                        if zb:
                            nc.vector.tensor_copy(dst, ps[:])
                        else:
                            nc.vector.tensor_scalar_add(
                                dst, ps[:], bqk_sb[:, bcol + t:bcol + t + 1])

                # ---- V projection for this block's 4 key tiles
                for st in range(4 * i, 4 * i + 4):
                    cc = st % 4
                    ps = ps_gen.tile([128, QB], F32, tag="psg")
                    for c in range(DKT):
                        nc.tensor.matmul(
                            ps[:, 0:HPC * DK], xT[:, i, c, cc * 128:(cc + 1) * 128],
                            wv_sb[:, c, :], start=(c == 0), stop=(c == DKT - 1))
                    nc.vector.tensor_copy(
                        V_sb[:, st, :, 0:DK],
                        ps[:, 0:HPC * DK].rearrange("p (h d) -> p h d", d=DK))

                # ---- attention + output projection for query block i
                ctx = ctx_pool.tile([DK, HPC, QB], F16, tag="ctx")
                c01 = ctx_pool.tile([128, QB], F16, tag="c01")
                if not causal:
                    mtile = msk_pool.tile([128, NKT, QB], F16, tag="mt")
                    nc.sync.dma_start(
                        mtile[:],
                        mTd.rearrange("(kt p) q -> p kt q", p=128)[:, :, i * QB:(i + 1) * QB])
                kt = 4 * (i + 1) if causal else NKT

                # valid column start for key tile j in query block i (causal)
                def lo(j):
                    return 128 * (j - 4 * i) if (causal and j >= 4 * i) else 0

                qcols = slice(i * QB, (i + 1) * QB)
                # QK^T row-packed: h0 on array rows 0-63, h1 on rows 64-127,
                # concurrent (disjoint row groups -> separate PSUM banks).  h2
                # packs pairs of its own k-tiles the same way via the duplicated
                # qT1/kT1 partition halves.
                pts = [pt_pool.tile([128, NKT, QB], F16, tag=f"pt{h}",
                                    name=f"pt{h}") for h in range(HPC)]
                for g2 in range(kt // 2):
                    # pure-diagonal pair group: only cols 256: are ever used
                    elo = 256 if (causal and g2 == 2 * i + 1) else 0
                    scA = ps_sc.tile([128, 2, QB], F32, tag="sc")
                    scB = ps_sc.tile([128, 2, QB], F32, tag="sc")
                    if "qk" in skip:
                        nc.vector.memset(scA[:, 0, 0:8], 1.0)
                        nc.vector.memset(scB[:, 0, 0:8], 1.0)
                    for jj in range(2 if "qk" not in skip else 0):
                        j = 2 * g2 + jj
                        l = lo(j)
                        nc.tensor.matmul(
                            scA[:, jj, l:QB], kT0[0:DK, j * 128:(j + 1) * 128],
                            qT0[0:DK, i * QB + l:(i + 1) * QB], start=True,
                            stop=True, tile_position=(0, 0))
                        nc.tensor.matmul(
                            scB[:, jj, l:QB], kT0[DK:128, j * 128:(j + 1) * 128],
                            qT0[DK:128, i * QB + l:(i + 1) * QB], start=True,
                            stop=True, tile_position=(DK, 0))
                    if "exp" in skip:
                        nc.vector.tensor_copy(pts[0][:, 2 * g2, 0:8], scA[:, 0, 0:8])
                        nc.vector.tensor_copy(pts[1][:, 2 * g2, 0:8], scB[:, 0, 0:8])
                    else:
                        nc.scalar.activation(
                            pts[0][:, 2 * g2:2 * g2 + 2, elo:QB],
                            scA[:, :, elo:QB], mybir.ActivationFunctionType.Exp)
                        nc.scalar.activation(
                            pts[1][:, 2 * g2:2 * g2 + 2, elo:QB],
                            scB[:, :, elo:QB], mybir.ActivationFunctionType.Exp)
                for g2 in range(kt // 2):
                    elo = 256 if (causal and g2 == 2 * i + 1) else 0
                    scC = ps_sc.tile([128, 2, QB], F32, tag="sc")
                    j0, j1 = 2 * g2, 2 * g2 + 1
                    if "qk" in skip:
                        nc.vector.memset(scC[:, 0, 0:8], 1.0)
                        continue
                    l0, l1 = lo(j0), lo(j1)
                    nc.tensor.matmul(
                        scC[:, 0, l0:QB], kT1[0:DK, j0 * 128:(j0 + 1) * 128],
                        qT1[0:DK, i * QB + l0:(i + 1) * QB], start=True,
                        stop=True, tile_position=(0, 0))
                    nc.tensor.matmul(
                        scC[:, 1, l1:QB], kT1[DK:128, j1 * 128:(j1 + 1) * 128],
                        qT1[DK:128, i * QB + l1:(i + 1) * QB], start=True,
                        stop=True, tile_position=(DK, 0))
                    if "exp" in skip:
                        nc.vector.tensor_copy(pts[2][:, 2 * g2, 0:8], scC[:, 0, 0:8])
                    else:
                        nc.scalar.activation(
                            pts[2][:, 2 * g2:2 * g2 + 2, elo:QB],
                            scC[:, :, elo:QB], mybir.ActivationFunctionType.Exp)
                    if "calib" in skip:
                        for _cb in range(4):
                            nc.scalar.activation(pts[2][:, 2 * g2:2 * g2 + 2, :],
                                                 scC[:],
                                                 mybir.ActivationFunctionType.Exp)
                # rowsums of all 3 heads collected onto partitions {0,32,64}
                # of one tile (via small cross-partition DMAs) -> ONE batched
                # reciprocal per query block instead of three
                rsc = rs_pool.tile([128, QB], F32, tag="rsc")
                rsc2 = rs_pool.tile([128, QB], F32, tag="rsc2")
                for h in range(HPC):
                    pt = pts[h]
                    if "mask" in skip:
                        pass
                    elif causal:
                        for jj in range(4):
                            j = 4 * i + jj
                            off = 128 * jj
                            nc.gpsimd.tensor_mul(
                                pt[:, j, off:off + 128], pt[:, j, off:off + 128],
                                tril[:])
                    else:
                        for j in range(NKT):
                            nc.vector.tensor_mul(
                                pt[:, j, :], pt[:, j, :], mtile[:, j, :])

                    pv = ps_pv.tile([128, QB], F32, tag="pv")
                    if "pv" in skip:
                        nc.vector.memset(pv[:, 0:8], 1.0)
                    for j in range(kt if "pv" not in skip else 0):
                        off = 128 * (j - 4 * i) if (causal and j >= 4 * i) else 0
                        nc.tensor.matmul(
                            pv[0:DK + 1, off:QB], V_sb[:, j, h, :],
                            pt[:, j, off:QB],
                            start=(j == 0), stop=(j == kt - 1),
                            skip_group_check=True)

                    dst = c01[0:DK, :] if h == 0 else ctx[:, h, :]
                    if "div" in skip:
                        nc.vector.tensor_copy(dst[:, 0:8], pv[0:DK, 0:8])
                    else:
                        rst = rs_pool.tile([128, QB], F32, tag="rst")
                        nc.scalar.copy(rst[DK:DK + 1, :], pv[DK:DK + 1, :])
                        nc.sync.dma_start(rsc[32 * h:32 * h + 1, :],
                                          rst[DK:DK + 1, :])
                        nc.vector.tensor_copy(dst, pv[0:DK, :])

                if "div" not in skip:
                    nc.vector.reciprocal(rsc2[0:DK + 1, :], rsc[0:DK + 1, :])
                    for h in range(HPC):
                        bc = ps_gen.tile([128, QB], F32, tag="psg")
                        nc.tensor.matmul(bc[0:DK, :], ones[32 * h:32 * h + 1, 0:DK],
                                         rsc2[32 * h:32 * h + 1, :],
                                         start=True, stop=True,
                                         tile_position=(32 * h, 0))
                        dst = c01[0:DK, :] if h == 0 else ctx[:, h, :]
                        nc.vector.tensor_mul(dst, dst, bc[0:DK, :])
                        if not zb:
                            nc.vector.tensor_scalar_add(dst, dst, bv_sb[:, h:h + 1])

                # h1 ctx shifted to partitions 64-127 via a small on-chip DMA
                # (DVE cannot cross partitions) -> 128-row contraction below
                nc.sync.dma_start(c01[DK:128, :], ctx[:, 1, :])
                osb = out_pool.tile([128, QB // 128, D], F16, tag="osb")
                for cch in range(QB // 128):
                    csl = slice(cch * 128, (cch + 1) * 128)
                    for nb, ncols in ((0, 512), (512, 256)):
                        ps = ps_gen.tile([128, QB], F32, tag="psg")
                        nc.tensor.matmul(ps[:, 0:ncols], c01[:, csl],
                                         wo_sb2[:, nb:nb + ncols],
                                         start=True, stop=False)
                        nc.tensor.matmul(ps[:, 0:ncols], ctx[:, 2, csl],
                                         wo_sb[:, nb:nb + ncols],
                                         start=False, stop=True)
                        if "dvemove" in skip:
                            nc.scalar.copy(osb[:, cch, nb:nb + ncols],
                                           ps[:, 0:ncols])
                        else:
                            nc.vector.tensor_copy(osb[:, cch, nb:nb + ncols],
                                                  ps[:, 0:ncols])
                if "odma" not in skip:
                    nc.sync.dma_start(
                        o16.rearrange("(i c p) d -> p i c d", p=128,
                                      c=QB // 128)[:, i], osb[:])

        pools = [cst, ps_gen, ps_sc, ps_pv, pt_pool, ctx_pool, rs_pool, out_pool]
        if not causal:
            pools.append(msk_pool)
        for p in reversed(pools):
            p.release()

    split_multi_waits(nc)
    return nc


_CACHE = {}


def _get_nc(causal, zb):
    key = (causal, zb)
    if key not in _CACHE:
        _CACHE[key] = build_nc(causal, zb)
    return _CACHE[key]


def _core_inputs(x, mask, wq, bq, wk, bk, wv, bv, wo, causal):
    ins = []
    wq8 = (wq * 0.125).astype(np.float32)
    bq8 = (bq * 0.125).astype(np.float32)
    if not causal:
        mT = (mask[0, 0].T != 0).astype(np.float16)

    # x pre-transposed per batch: [768, 2048] -> chunk-major (NQB,128,DKT,QB)
    xts = []
    for b in range(B):
        xt = np.ascontiguousarray(x[b].T).astype(np.float16)
        xts.append(np.ascontiguousarray(
            xt.reshape(DKT, 128, NQB, QB).transpose(2, 1, 0, 3)))

    def pack_w(w):
        # [768, M] -> dense per-partition [128, DKT, M]
        return np.ascontiguousarray(
            w.reshape(DKT, 128, -1).transpose(1, 0, 2)).astype(np.float16)

    for core in range(N_CORES):
        b, g = divmod(core, 4)
        hs = [HPC * g + k for k in range(HPC)]
        cols = lambda w, h: w[:, h * DK:(h + 1) * DK]

        wqe = pack_w(np.concatenate(
            [cols(wq8, hs[0]), cols(wq8, hs[1]), cols(wq8, hs[2]),
             cols(wq8, hs[2])], axis=1))
        wke = pack_w(np.concatenate(
            [cols(wk, hs[0]), cols(wk, hs[1]), cols(wk, hs[2]), cols(wk, hs[2])],
            axis=1))
        wve = pack_w(np.concatenate([cols(wv, h) for h in hs], axis=1))
        woh2 = wo[hs[2] * DK:(hs[2] + 1) * DK].astype(np.float16)
        woe2 = np.concatenate([wo[hs[0] * DK:(hs[0] + 1) * DK],
                               wo[hs[1] * DK:(hs[1] + 1) * DK]]).astype(np.float16)
        seg = lambda v, h: v[h * DK:(h + 1) * DK]
        bqk_pack = np.stack([
            np.concatenate([seg(bq8, hs[0]), seg(bq8, hs[1])]),
            np.concatenate([seg(bq8, hs[2]), seg(bq8, hs[2])]),
            np.concatenate([seg(bk, hs[0]), seg(bk, hs[1])]),
            np.concatenate([seg(bk, hs[2]), seg(bk, hs[2])]),
        ], axis=1).astype(np.float32)
        bvp = np.stack([seg(bv, h) for h in hs], axis=1).astype(np.float32)

        m = {
            "x16t": xts[b],
            "wqe": wqe, "wke": wke, "wve": wve, "woh2": woh2, "woe2": woe2,
            "bqk": bqk_pack, "bvp": bvp,
        }
        if not causal:
            m["mT"] = mT
        ins.append(m)
    return ins


def kernel(x, mask, wq, bq, wk, bk, wv, bv, wo, bo):
    x = np.asarray(x)
    mask = np.asarray(mask)
    m2 = np.asarray(mask[0, 0])
    causal = bool(np.array_equal(m2, np.tril(np.ones((S, S), m2.dtype))))
    bq, bk, bv = np.asarray(bq), np.asarray(bk), np.asarray(bv)
    zb = bool(not bq.any() and not bk.any() and not bv.any())
    nc = _get_nc(causal, zb)
    ins = _core_inputs(x, mask, np.asarray(wq), bq, np.asarray(wk),
                       bk, np.asarray(wv), bv, np.asarray(wo), causal)
    res = run_bass_kernel_spmd(nc, ins, core_ids=list(range(N_CORES)))
    out = np.zeros((B, S, D), np.float32)
    for core in range(N_CORES):
        b = core // 4
        out[b] += res.results[core]["o16"].astype(np.float32)
    out += np.asarray(bo, np.float32)
    return out
